# revision 1
# baseline (speedup 1.0000x reference)
"""Trainium2 Bass kernel for nn_CountingAbstraction (B=4, N=D=2048).

Math (per example):
    cn   = l2_normalize(data, axis=-1)
    sim  = relu(cn @ cn.T)                      # [N, N], symmetric
    v    = posenc @ sim                         # [N, N]
    csum = sim.sum(-1)                          # [N]
    counter = softplus(concat([csum, v], -1) @ W_exp + b_exp)
    out  = concat([data, counter], -1) @ W_merge

Decomposition used on device: with S0 = dataT.T @ dataT (raw gram of the
bf16-cast data), diag(S0) gives the squared row norms, so with
Dn = diag(rsqrt(diag(S0))) and R = relu(S0):
    sim = Dn R Dn
    v[n, :]   = inv_m * (posenc Dn R)[n, :]     (computed transposed: vT[m, n])
    csum[m]   = inv_m * sum_k R[k, m] inv_k
The two Dense layers then run entirely in "transposed" ([feature, n]) layout,
which makes every matmul in the chain consume the previous stage's output as
the moving operand with zero transposes.

Sharding: 8 cores; core c handles example b = c//2 and output-row half
h = c % 2 (rows [h*1024, (h+1)*1024)). Each core computes the full N x N
gram for its example (compute is redundant 2x on that stage but requires no
cross-core communication). All matmul operands are bf16 with fp32 PSUM
accumulation; the final output is fp32.

The host side only does layout work: transposes, bf16 casts, weight
reordering/swizzling, and computing the (constant) positional encoding.
All input-dependent math runs on device.
"""

import numpy as np
from ml_dtypes import bfloat16

import concourse.bass as bass
import concourse.mybir as mybir
import concourse.tile as tile
from concourse.vector_clock import ScopedClock
from concourse.bass_utils import run_bass_kernel_spmd

P = 128
FREE = 512  # max matmul moving free dim (one PSUM bank of fp32)
EPS = 1e-12
F32 = mybir.dt.float32
BF16 = mybir.dt.bfloat16


class SplitDrainTileContext(tile.TileContext):
    """Walrus rejects >1 sync-wait on the kernel-tail Drain; split the waits
    across a chain of single-wait drains on the sync engine."""

    MAX_WAITS = 1

    def _drain_and_barrier(self, tick_clock, wait_clock):
        drain_inst = self.nc.sync.drain()
        wait_clock.add_sem_waits(
            drain_inst.ins, ScopedClock({None: tick_clock.global_clock})
        )
        si = drain_inst.ins.sync_info
        if si is not None and len(si.on_wait) > self.MAX_WAITS:
            waits = list(si.on_wait)
            drain_inst.ins.sync_info = mybir.SyncInfo(
                on_wait=waits[: self.MAX_WAITS], on_update=list(si.on_update)
            )
            for i in range(self.MAX_WAITS, len(waits), self.MAX_WAITS):
                extra = self.nc.sync.drain()
                extra.ins.sync_info = mybir.SyncInfo(
                    on_wait=waits[i : i + self.MAX_WAITS], on_update=[]
                )

        self.nc.all_engine_barrier()
        assert self.sems is not None
        popped = self.nc._tile_sem_poison_stack.pop()
        assert popped is self._sem_poison
        self.nc.clear_and_free_semaphores(list(self.sems.allocated().values()))
        self.nc.all_engine_barrier()


def _free_chunks(total):
    return [(s, min(FREE, total - s)) for s in range(0, total, FREE)]


def _split_multi_waits(nc, max_waits=1):
    """This walrus build rejects instructions carrying more than one sync
    wait. Hoist extra waits into standalone InstEventSemaphore instructions
    inserted just before the instruction in its engine's stream."""
    n_new = 0
    for fn in nc.m.functions:
        for blk in fn.blocks:
            new_insts = []
            for inst in blk.instructions:
                si = inst.sync_info
                if si is not None and len(si.on_wait) > max_waits:
                    waits = list(si.on_wait)
                    for w in waits[max_waits:]:
                        es = mybir.InstEventSemaphore(
                            name=f"I-hoistw-{n_new}", ins=[], outs=[]
                        )
                        es.engine = inst.engine
                        es.sync_info = mybir.SyncInfo(on_wait=[w], on_update=[])
                        new_insts.append(es)
                        n_new += 1
                    inst.sync_info = mybir.SyncInfo(
                        on_wait=waits[:max_waits], on_update=list(si.on_update)
                    )
                new_insts.append(inst)
            blk.instructions = new_insts
    return n_new


def build_program(N, D, NB, debug=False):
    """Emit the SPMD per-core program. N == D, NB = N // 2 (row half)."""
    assert N == D and NB * 2 == N
    NT = N // P          # contraction / row tiles
    QT = NT + 1          # W_exp subtiles (last one carries csum rows)
    CT = 2 * NT          # merge contraction tiles
    ET = NT              # output feature tiles
    NBT = NB // P

    nc = bass.Bass("TRN2")

    dataT = nc.dram_tensor("dataT", [N, N], BF16, kind="ExternalInput")
    dataTnb = nc.dram_tensor("dataTnb", [N, NB], BF16, kind="ExternalInput")
    pdTr = nc.dram_tensor("pdTr", [N, NB], BF16, kind="ExternalInput")
    wexp = nc.dram_tensor("wexp", [NT, QT, P, P], BF16, kind="ExternalInput")
    wm = nc.dram_tensor("wm", [ET, CT, P, P], BF16, kind="ExternalInput")
    bexp = nc.dram_tensor("bexp", [D], F32, kind="ExternalInput")
    ident = nc.dram_tensor("ident", [P, P], F32, kind="ExternalInput")
    outT = nc.dram_tensor("outT", [D, NB], F32, kind="ExternalOutput")
    if debug:
        d_nrm2 = nc.dram_tensor("d_nrm2", [N], F32, kind="ExternalOutput")
        d_inv = nc.dram_tensor("d_inv", [N], F32, kind="ExternalOutput")
        d_csum = nc.dram_tensor("d_csum", [2, NB], F32, kind="ExternalOutput")
        d_R = nc.dram_tensor("d_R", [N, N], BF16, kind="ExternalOutput")
        d_pdT = nc.dram_tensor("d_pdT", [N, NB], BF16, kind="ExternalOutput")
        d_vT = nc.dram_tensor("d_vT", [N, NB], BF16, kind="ExternalOutput")
        d_ct = nc.dram_tensor("d_ct", [D, NB], BF16, kind="ExternalOutput")

    dataT_t = dataT.rearrange("(o p) f -> p o f", p=P)
    dataTnb_t = dataTnb.rearrange("(o p) f -> p o f", p=P)
    pdTr_t = pdTr.rearrange("(o p) f -> p o f", p=P)
    bexp_t = bexp.rearrange("(o p) -> p o", p=P)
    outT_t = outT.rearrange("(o p) f -> p o f", p=P)

    with SplitDrainTileContext(nc) as tc:
        with (
            tc.tile_pool(name="big", bufs=1) as big,
            tc.tile_pool(name="small", bufs=1) as small,
            tc.tile_pool(name="wstream", bufs=2) as wstream,
            tc.tile_pool(name="evict", bufs=2) as evict,
            tc.tile_pool(name="ps", bufs=2, space="PSUM") as ps,
            tc.tile_pool(name="dram", bufs=1, space="DRAM") as dram,
        ):
            # ---- resident tensors -------------------------------------
            # Two 64KB/partition slots shared by lifetime chains:
            #   slot A: dT (st0-1) -> pdT (st2-3) -> dnb (st4-5)
            #   slot B: R (st1-3) -> counterT (st4-5)
            # vT gets its own 32KB slot. Max concurrency per tag == bufs.
            dT = big.tile([P, NT, N], BF16, tag="huge", bufs=2)
            R = big.tile([P, NT, N], BF16, tag="huge", bufs=2)
            pdT = big.tile([P, NT, NB], BF16, tag="huge", bufs=2)
            vT = big.tile([P, NT, NB], BF16, tag="med", bufs=1)
            counterT = big.tile([P, NT, NB], BF16, tag="huge", bufs=2)
            dnb = big.tile([P, NT, NB], BF16, tag="huge", bufs=2)

            ident_sb = small.tile([P, P], F32, tag="ident")
            bexp_sb = small.tile([P, NT], F32, tag="bexp")
            nrm2 = small.tile([P, NT], F32, tag="nrm2")
            nrm = small.tile([P, NT], F32, tag="nrm")
            inv = small.tile([P, NT], F32, tag="inv")
            inv_bf = small.tile([P, NT], BF16, tag="inv_bf")
            csum_col = small.tile([P, NT], F32, tag="csum_col")
            csum_rows = small.tile([2, NB], F32, tag="csum_rows")
            aug = small.tile([P, NB], BF16, tag="aug")
            diag_tmp = small.tile([P, P], F32, tag="diag_tmp")

            dram_csum = dram.tile([N], F32)

            # ---- input DMAs (chunked across queues) -------------------
            nc.sync.dma_start(ident_sb[:], ident[:])
            nc.sync.dma_start(bexp_sb[:], bexp_t)
            lc = max(1, NT // 8)
            for o in range(0, NT, lc):
                nc.sync.dma_start(dT[:, o : o + lc, :], dataT_t[:, o : o + lc, :])

            # ---- stage 1: gram S0 = dT.T @ dT, relu -> R, diag -> norms
            for it in range(NT):
                psg = ps.tile([P, 2048], F32, tag="ps")
                for dt_ in range(NT):
                    lhsT = dT[:, dt_, it * P : (it + 1) * P]
                    for (s, w) in _free_chunks(N):
                        nc.tensor.matmul(
                            psg[:, s : s + w],
                            lhsT,
                            dT[:, dt_, s : s + w],
                            start=(dt_ == 0),
                            stop=(dt_ == NT - 1),
                        )
                # diagonal block -> squared norms for this row tile
                nc.vector.tensor_tensor(
                    diag_tmp[:],
                    psg[:, it * P : (it + 1) * P],
                    ident_sb[:],
                    mybir.AluOpType.mult,
                )
                nc.vector.reduce_sum(
                    nrm2[:, it : it + 1], diag_tmp[:], axis=mybir.AxisListType.X
                )
                nc.scalar.activation(
                    R[:, it, :], psg[:, :N], mybir.ActivationFunctionType.Relu
                )

            # pdT load deferred past the gram: it reuses dT's SBUF slot, and
            # emitting it earlier would block its DMA queue on the slot wait.
            for o in range(0, NT, lc):
                nc.sync.dma_start(pdT[:, o : o + lc, :], pdTr_t[:, o : o + lc, :])

            # ---- inv = rsqrt(max(nrm2, eps)) --------------------------
            nc.vector.tensor_scalar_max(nrm2[:], nrm2[:], EPS)
            nc.scalar.sqrt(nrm[:], nrm2[:])
            nc.vector.reciprocal(inv[:], nrm[:])
            nc.vector.tensor_copy(inv_bf[:], inv[:])

            # pdT = pdTr * inv_k  (per-partition scale, row tile by row tile)
            for kt in range(NT):
                nc.vector.tensor_scalar_mul(
                    pdT[:, kt, :], pdT[:, kt, :], inv[:, kt : kt + 1]
                )

            # ---- stage 3: vT[m, n] = inv_m * sum_k R[k, m] pdT[k, n]
            #      csum_col[m] = inv_m * sum_k R[k, m] inv_bf[k]
            # csum column must live in its own PSUM bank: start=True resets
            # the whole bank's has_written bits, so an interleaved second
            # accumulation group sharing a bank wipes the first's partials.
            CS_OFF = ((NB + FREE - 1) // FREE) * FREE
            for mt in range(NT):
                psv = ps.tile([P, 2048], F32, tag="ps")
                for kt in range(NT):
                    lhsT = R[:, kt, mt * P : (mt + 1) * P]
                    for (s, w) in _free_chunks(NB):
                        nc.tensor.matmul(
                            psv[:, s : s + w],
                            lhsT,
                            pdT[:, kt, s : s + w],
                            start=(kt == 0),
                            stop=(kt == NT - 1),
                        )
                    nc.tensor.matmul(
                        psv[:, CS_OFF : CS_OFF + 1],
                        lhsT,
                        inv_bf[:, kt : kt + 1],
                        start=(kt == 0),
                        stop=(kt == NT - 1),
                    )
                nc.scalar.activation(
                    vT[:, mt, :],
                    psv[:, :NB],
                    mybir.ActivationFunctionType.Copy,
                    scale=inv[:, mt : mt + 1],
                )
                nc.scalar.activation(
                    csum_col[:, mt : mt + 1],
                    psv[:, CS_OFF : CS_OFF + 1],
                    mybir.ActivationFunctionType.Copy,
                    scale=inv[:, mt : mt + 1],
                )

            # csum column -> two row halves (via DRAM bounce), into aug rows
            nc.sync.dma_start(dram_csum.rearrange("(o p) -> p o", p=P), csum_col[:])
            nc.sync.dma_start(csum_rows[:], dram_csum.rearrange("(h n) -> h n", h=2))
            nc.vector.memset(aug[:], 0.0)
            nc.vector.tensor_copy(aug[0:2, :], csum_rows[:])

            # ---- stage 4: counterT[d, n] = softplus(
            #        sum_q wexp[q, d] * [vT; aug][q, n] + bexp[d])
            for dt_ in range(NT):
                wx = wstream.tile([P, QT, P], BF16, tag="wx")
                nc.sync.dma_start(wx[:], wexp[dt_].rearrange("q p f -> p q f"))
                psc = ps.tile([P, 2048], F32, tag="ps")
                for qt in range(QT):
                    lhsT = wx[:, qt, :]
                    rhs_tile = vT[:, qt, :] if qt < NT else aug[:, :]
                    for (s, w) in _free_chunks(NB):
                        nc.tensor.matmul(
                            psc[:, s : s + w],
                            lhsT,
                            rhs_tile[:, s : s + w],
                            start=(qt == 0),
                            stop=(qt == QT - 1),
                        )
                # softplus(x + b) as ln(exp(x + b) + 1): walrus has no
                # Softplus LUT set; Exp/Ln are exact enough (~1e-5) here.
                spt = evict.tile([P, NB], F32, tag="spt")
                nc.scalar.activation(
                    spt[:],
                    psc[:, :NB],
                    mybir.ActivationFunctionType.Exp,
                    bias=bexp_sb[:, dt_ : dt_ + 1],
                )
                nc.scalar.activation(
                    counterT[:, dt_, :],
                    spt[:],
                    mybir.ActivationFunctionType.Ln,
                    bias=1.0,
                )

            # dnb load deferred likewise (reuses pdT's slot after stage 3)
            for o in range(0, NT, lc):
                nc.sync.dma_start(dnb[:, o : o + lc, :], dataTnb_t[:, o : o + lc, :])

            # ---- stage 5: outT[e, n] = sum_c wm[c, e] * [dnb; counterT][c, n]
            for et in range(ET):
                pso = ps.tile([P, 2048], F32, tag="ps")
                for h2 in range(2):
                    wmt = wstream.tile([P, NT, P], BF16, tag="wmt")
                    nc.sync.dma_start(
                        wmt[:],
                        wm[et, h2 * NT : (h2 + 1) * NT].rearrange("c p f -> p c f"),
                    )
                    for ci in range(NT):
                        ct = h2 * NT + ci
                        lhsT = wmt[:, ci, :]
                        rhs_tile = (
                            dnb[:, ct, :] if ct < NT else counterT[:, ct - NT, :]
                        )
                        for (s, w) in _free_chunks(NB):
                            nc.tensor.matmul(
                                pso[:, s : s + w],
                                lhsT,
                                rhs_tile[:, s : s + w],
                                start=(ct == 0),
                                stop=(ct == CT - 1),
                            )
                osb = evict.tile([P, NB], F32, tag="osb")
                nc.vector.tensor_copy(osb[:], pso[:, :NB])
                nc.sync.dma_start(outT_t[:, et, :], osb[:])

            if debug:
                nc.sync.dma_start(
                    d_nrm2.rearrange("(o p) -> p o", p=P), nrm2[:]
                )
                nc.sync.dma_start(d_inv.rearrange("(o p) -> p o", p=P), inv[:])
                nc.sync.dma_start(d_csum[:], csum_rows[:])
                nc.sync.dma_start(
                    d_R.rearrange("(o p) f -> p o f", p=P), R[:]
                )
                nc.sync.dma_start(
                    d_pdT.rearrange("(o p) f -> p o f", p=P), pdT[:]
                )
                nc.sync.dma_start(
                    d_vT.rearrange("(o p) f -> p o f", p=P), vT[:]
                )
                nc.sync.dma_start(
                    d_ct.rearrange("(o p) f -> p o f", p=P), counterT[:]
                )

    _split_multi_waits(nc)
    return nc


# ---------------------------------------------------------------------------
# host side
# ---------------------------------------------------------------------------

def get_posenc(n, d):
    pos = np.arange(n)[:, None].astype(np.float32)
    i = np.arange(d)[None, :]
    angle_rates = 1.0 / np.power(
        10000.0, (2 * (i // 2)).astype(np.float32) / np.float32(d)
    )
    angles = pos * angle_rates
    pe = np.zeros((n, d), dtype=np.float32)
    pe[:, 0::2] = np.sin(angles[:, 0::2])
    pe[:, 1::2] = np.cos(angles[:, 1::2])
    return pe


def _host_prep(data, W_exp, b_exp, W_merge):
    """Layout-only host prep; returns per-core input maps."""
    B, N, D = data.shape
    NB = N // 2
    NT = N // P
    QT = NT + 1
    CT = 2 * NT
    ET = NT

    posenc = get_posenc(N, D)
    pT = posenc.T.astype(bfloat16)  # [k, n]

    # W_exp padded/swizzled per half h: rows 0..N-1 = W_exp[1:], row N+h = W_exp[0]
    wexp_v = []
    for h in range(2):
        wpad = np.zeros((QT * P, D), dtype=bfloat16)
        wpad[:N] = W_exp[1:].astype(bfloat16)
        wpad[N + h] = W_exp[0].astype(bfloat16)
        wexp_v.append(
            np.ascontiguousarray(
                wpad.reshape(QT, P, NT, P).transpose(2, 0, 1, 3)
            )
        )  # [dt, qt, p, f]

    wm_s = np.ascontiguousarray(
        W_merge.astype(bfloat16).reshape(CT, P, ET, P).transpose(2, 0, 1, 3)
    )  # [et, ct, p, f]

    bexp_f = np.ascontiguousarray(b_exp.astype(np.float32))
    ident = np.eye(P, dtype=np.float32)

    dataT_b = [np.ascontiguousarray(data[b].T.astype(bfloat16)) for b in range(B)]

    in_maps = []
    for c in range(2 * B):
        b, h = c // 2, c % 2
        nb = slice(h * NB, (h + 1) * NB)
        in_maps.append(
            {
                "dataT": dataT_b[b],
                "dataTnb": np.ascontiguousarray(dataT_b[b][:, nb]),
                "pdTr": np.ascontiguousarray(pT[:, nb]),
                "wexp": wexp_v[h],
                "wm": wm_s,
                "bexp": bexp_f,
                "ident": ident,
            }
        )
    return in_maps


_program_cache = {}


def _get_program(N, D, NB):
    key = (N, D, NB)
    if key not in _program_cache:
        _program_cache[key] = build_program(N, D, NB)
    return _program_cache[key]


def kernel(data, W_exp, b_exp, W_merge):
    data = np.asarray(data)
    W_exp = np.asarray(W_exp)
    b_exp = np.asarray(b_exp)
    W_merge = np.asarray(W_merge)
    B, N, D = data.shape
    NB = N // 2

    nc = _get_program(N, D, NB)
    in_maps = _host_prep(data, W_exp, b_exp, W_merge)
    core_ids = list(range(2 * B))
    res = run_bass_kernel_spmd(nc, in_maps, core_ids)

    out = np.empty((B, N, D), dtype=np.float32)
    for c in core_ids:
        b, h = c // 2, c % 2
        out[b, h * NB : (h + 1) * NB, :] = res.results[c]["outT"].T
    return out

